# revision 30
# baseline (speedup 1.0000x reference)
"""Trainium2 Bass kernel for GQA attention (32 q heads / 16 kv heads, head_dim
128, L=2048, D=4608) with RoPE, tanh softcap 50, causal mask, o_proj.

Tensor-parallel over heads across 8 NeuronCores; core c owns q-heads 4c..4c+3
and kv-heads 2c..2c+1; host sums the 8 partial [L, D] outputs.

Phase 1: one pass over x columns per 512-wide chunk; 512-wide Q/K chains
(LDWEIGHTS fully hidden under the 213ns matmuls), 256-wide V chains; rope
applied during the PSUM drain; wq[k]/xc[k] DMAs interleaved so the first
chain starts ~10us in.  V carries a ones column per k-tile ([128, 16*129])
so the PV matmul accumulates the softmax denominator for free.

Phase 2 schedule (the win over the naive per-head loop, 611us -> 544us):
  - scores are computed in PAIRS of k-tiles: two 512-wide score MMs land in
    one 2-bank PSUM tile, then a single tanh and a single exp cover
    [128, 1024], halving the scalar-engine instruction count (its 352-cycle
    per-op overhead was a third of the softcap cost).  Diagonal tiles write
    q-aligned into the pair so one exp keeps columns aligned; tanh/exp of the
    stale PSUM in masked gaps is bounded and memset-zeroed afterwards.
  - PV for head h-1 (129-wide accumulation chains incl. the denominator
    column, then reciprocal/scale/PE-transpose) is interleaved unit-by-unit
    with head h's score pairs, so the PE always has ready matmuls while the
    scalar engine chews the tanh+exp backlog.
  - o_proj (s, j)-groups are drawn from a deferred queue with per-chunk
    budgets {0, 20, 32, rest}: later chunks have longer softcap chains, so
    more PE filler is reserved for them.
"""
import numpy as np
import ml_dtypes

import concourse.bass as bass
import concourse.mybir as mybir
import concourse.tile as tile
from concourse import bacc

F32 = mybir.dt.float32
BF16 = mybir.dt.bfloat16
BF16_NP = ml_dtypes.bfloat16
AF = mybir.ActivationFunctionType

N_HEADS = 32
N_KV = 16
HEAD_DIM = 128
ROPE_THETA = 10000.0
SOFTCAP = 50.0
SCALE = 1.0 / 12.0  # 1/sqrt(144)
L = 2048
D = 4608
N_CORES = 8
QH = N_HEADS // N_CORES        # 4 local q heads
KVH = N_KV // N_CORES          # 2 local kv heads
KC = D // 128                  # 36 contraction chunks
NQ = L // 512                  # 4 l-chunks of 512
LT = L // 128                  # 16 l-tiles of 128
DOUT_CHUNKS = D // 512         # 9 o_proj output chunks
PAIR_LAG = 1                   # attnT MM pair trails the exp by this many pairs


def _emit(nc):
    xt_d = nc.dram_tensor("xt", [NQ * KC * 128, 512], BF16, kind="ExternalInput")
    wqt_d = nc.dram_tensor("wqt", [D, QH * 128], BF16, kind="ExternalInput")
    wkt_d = nc.dram_tensor("wkt", [D, KVH * 128], BF16, kind="ExternalInput")
    wvt_d = nc.dram_tensor("wvt", [D, KVH * 128], BF16, kind="ExternalInput")
    wot_d = nc.dram_tensor("wot", [QH * 128, D], BF16, kind="ExternalInput")
    cost_d = nc.dram_tensor("cost", [128, L], BF16, kind="ExternalInput")
    sint_d = nc.dram_tensor("sint", [128, L], BF16, kind="ExternalInput")
    mask_d = nc.dram_tensor("mask", [128, 512], BF16, kind="ExternalInput")
    out_d = nc.dram_tensor("out", [L, D], F32, kind="ExternalOutput")

    with tile.TileContext(nc) as tc:
        with (
            tc.tile_pool(name="const", bufs=1) as const,
            tc.tile_pool(name="persist", bufs=1) as persist,
        ):
            ones = const.tile([128, 1], F32)
            nc.vector.memset(ones[:], 1.0)
            warm = const.tile([128, 2], F32)
            # preload the exp/tanh activation table set during phase 1
            nc.scalar.activation(warm[:, 0:1], ones[:], AF.Tanh)
            nc.scalar.activation(warm[:, 1:2], warm[:, 0:1], AF.Exp)
            cost = const.tile([128, L], BF16)
            sint = const.tile([128, L], BF16)
            mask0 = const.tile([128, 512], BF16)

            # persistent per-head tensors
            QT = [persist.tile([128, L], BF16, tag=f"qt{h}", name=f"qt{h}") for h in range(QH)]
            KT = [persist.tile([128, L], BF16, tag=f"kt{g}", name=f"kt{g}") for g in range(KVH)]
            # V extended with a ones column per k-tile: [128, 16*129]
            V = [persist.tile([128, LT * 129], BF16, tag=f"v{g}", name=f"v{g}") for g in range(KVH)]

            _phase1(nc, tc, cost, sint, mask0,
                    dict(xt=xt_d, wqt=wqt_d, wkt=wkt_d, wvt=wvt_d,
                         cost=cost_d, sint=sint_d, mask=mask_d), QT, KT, V)
            _phase2(nc, tc, ones, mask0, QT, KT, V, wot_d, out_d)
    return nc


def _phase1(nc, tc, cost, sint, mask0, dram, QT, KT, V):
    def drain_rope(rtmp, ps, dst, nq, pj_free):
        """psum [128,512] f32 -> rope -> dst bf16 [128,512] slice."""
        cols = slice(nq * 512, (nq + 1) * 512)
        raw = rtmp.tile([128, 512], F32, tag="raw", name="raw")
        nc.scalar.activation(raw[:], ps[:], AF.Copy)
        swap = rtmp.tile([128, 512], F32, tag="swap", name="swap")
        nc.scalar.activation(swap[0:64, :], ps[64:128, :], AF.Copy)
        nc.scalar.activation(swap[64:128, :], ps[0:64, :], AF.Copy)
        nc.vector.tensor_mul(raw[:], raw[:], cost[:, cols])
        nc.vector.tensor_mul(swap[:], swap[:], sint[:, cols])
        nc.vector.tensor_add(dst[:, cols], raw[:], swap[:])

    with (
        tc.tile_pool(name="xcol", bufs=2) as xcol,
        tc.tile_pool(name="rtmp", bufs=3) as rtmp,
        tc.tile_pool(name="wts", bufs=1) as wts,
        tc.tile_pool(name="pj_psum", bufs=2, space="PSUM") as pj_psum,
        tc.tile_pool(name="warm_ps", bufs=1, space="PSUM") as warm_ps,
    ):
        # single pass over x columns computing Q, K (rope'd, [d, l]) and V
        # (direct [l, d] with xT stationary) per 512-wide chunk.
        wq, wk, wv, xc0 = [], [], [], []
        for k in range(KC):
            w = wts.tile([128, QH * 128], BF16, tag=f"q{k}", name=f"wq{k}")
            nc.sync.dma_start(w[:], dram["wqt"][k * 128:(k + 1) * 128, :])
            wq.append(w)
            t = xcol.tile([128, 512], BF16, tag=f"x{k}", name=f"xc{k}")
            nc.sync.dma_start(t[:], dram["xt"][k * 128:(k + 1) * 128, :])
            xc0.append(t)
            w = wts.tile([128, KVH * 128], BF16, tag=f"k{k}", name=f"wk{k}")
            nc.sync.dma_start(w[:], dram["wkt"][k * 128:(k + 1) * 128, :])
            wk.append(w)
        nc.sync.dma_start(cost[:], dram["cost"][:])
        nc.sync.dma_start(sint[:], dram["sint"][:])
        for k in range(KC):
            w = wts.tile([128, KVH * 128], BF16, tag=f"v{k}", name=f"wv{k}")
            nc.sync.dma_start(w[:], dram["wvt"][k * 128:(k + 1) * 128, :])
            wv.append(w)
        nc.sync.dma_start(mask0[:], dram["mask"][:])

        for nq in range(NQ):
            if nq == 0:
                # k-outer warmup: all 6 Q/K chains accumulate concurrently in
                # 6 PSUM banks, so each arriving (wq, x, wk) DMA trio feeds 6
                # matmuls - the cold-start is compute- not DMA-bound.
                xc = xc0
                wps = [warm_ps.tile([128, 512], F32, tag=f"w{i}", name=f"wps{i}")
                       for i in range(QH + KVH)]
                for k in range(KC):
                    for h in range(QH):
                        nc.tensor.matmul(
                            wps[h][:], wq[k][:, h * 128:(h + 1) * 128], xc[k][:],
                            start=(k == 0), stop=(k == KC - 1),
                            skip_group_check=True)
                    for g in range(KVH):
                        nc.tensor.matmul(
                            wps[QH + g][:], wk[k][:, g * 128:(g + 1) * 128], xc[k][:],
                            start=(k == 0), stop=(k == KC - 1),
                            skip_group_check=True)
                for h in range(QH):
                    drain_rope(rtmp, wps[h], QT[h], 0, pj_psum)
                for g in range(KVH):
                    drain_rope(rtmp, wps[QH + g], KT[g], 0, pj_psum)
            else:
                xc = []
                for k in range(KC):
                    t = xcol.tile([128, 512], BF16, tag=f"x{k}", name=f"xc{k}")
                    row = (nq * KC + k) * 128
                    nc.sync.dma_start(t[:], dram["xt"][row:row + 128, :])
                    xc.append(t)
                for h in range(QH):
                    ps = pj_psum.tile([128, 512], F32, tag="qk", name="ps")
                    for k in range(KC):
                        nc.tensor.matmul(
                            ps[:], wq[k][:, h * 128:(h + 1) * 128], xc[k][:],
                            start=(k == 0), stop=(k == KC - 1))
                    drain_rope(rtmp, ps, QT[h], nq, pj_psum)
                for g in range(KVH):
                    ps = pj_psum.tile([128, 512], F32, tag="qk", name="ps")
                    for k in range(KC):
                        nc.tensor.matmul(
                            ps[:], wk[k][:, g * 128:(g + 1) * 128], xc[k][:],
                            start=(k == 0), stop=(k == KC - 1))
                    drain_rope(rtmp, ps, KT[g], nq, pj_psum)
            for sub in range(4):
                mk = nq * 4 + sub
                ps = pj_psum.tile([128, 512], F32, tag="qk", name="ps")
                for k in range(KC):
                    nc.tensor.matmul(
                        ps[:, 0:KVH * 128], xc[k][:, sub * 128:(sub + 1) * 128],
                        wv[k][:], start=(k == 0), stop=(k == KC - 1))
                for g in range(KVH):
                    nc.vector.tensor_copy(
                        V[g][:, mk * 129:mk * 129 + 128],
                        ps[:, g * 128:(g + 1) * 128])
                    nc.vector.memset(
                        V[g][:, mk * 129 + 128:mk * 129 + 129], 1.0)


def _phase2(nc, tc, ones, mask0, QT, KT, VE, wot_d, out_d):
    from concourse.masks import make_identity

    with (
        tc.tile_pool(name="wo", bufs=1) as wop,
        tc.tile_pool(name="ident", bufs=1) as idp,
        tc.tile_pool(name="ttp", bufs=2) as ttp,
        tc.tile_pool(name="ptp", bufs=16) as ptp,
        tc.tile_pool(name="attnt", bufs=3) as attp,
        tc.tile_pool(name="small", bufs=4) as small,
        tc.tile_pool(name="ost", bufs=6) as ost,
        tc.tile_pool(name="op_ps", bufs=2, space="PSUM") as op_ps,
    ):
        ident = idp.tile([128, 128], BF16)
        make_identity(nc, ident[:])
        WO = []
        for h in range(QH):
            w = wop.tile([128, D], BF16, tag=f"wo{h}")
            nc.sync.dma_start(w[:], wot_d[h * 128:(h + 1) * 128, :])
            WO.append(w)

        at_store = {}

        def make_job(h, r):
            """Closures for one (head, chunk): score pairs then 4 PV units."""
            g = h // 2
            nkt = 4 * r + 4
            npair = nkt // 2
            qt = QT[h]
            q0 = r * 512
            pts = {}

            def pair_unit(p):
                sc = sc_ps.tile([128, 1024], F32, tag="sc", name="sc")
                c0s = []
                for i in range(2):
                    mk = 2 * p + i
                    o = mk - 4 * r
                    c0 = max(0, o) * 128
                    c0s.append(c0)
                    nc.tensor.matmul(sc[:, i * 512 + c0:(i + 1) * 512],
                                     KT[g][:, mk * 128:(mk + 1) * 128],
                                     qt[:, q0 + c0:q0 + 512],
                                     start=True, stop=True)
                tt = ttp.tile([128, 1024], F32, tag="tt", name="tt")
                pt = ptp.tile([128, 1024], BF16, tag="pt", name="pt")
                pts[p] = pt
                if c0s[0] + c0s[1] > 424:
                    # heavily-masked diagonal pair: exact-width ops beat one
                    # batched op (352-cycle ACT overhead vs dead columns)
                    for i in range(2):
                        a, b = i * 512 + c0s[i], (i + 1) * 512
                        nc.scalar.activation(tt[:, a:b], sc[:, a:b], AF.Tanh,
                                             scale=SCALE / SOFTCAP)
                        nc.scalar.activation(pt[:, a:b], tt[:, a:b], AF.Exp,
                                             scale=SOFTCAP)
                else:
                    # tanh of stale psum in masked gaps is bounded; its exp is
                    # finite and the memset below zeroes it.
                    nc.scalar.activation(tt[:], sc[:], AF.Tanh, scale=SCALE / SOFTCAP)
                    nc.scalar.activation(pt[:], tt[:], AF.Exp, scale=SOFTCAP)
                for i in range(2):
                    mk = 2 * p + i
                    o = mk - 4 * r
                    c0 = max(0, o) * 128
                    base = i * 512
                    if o >= 0:
                        if c0 > 0:
                            nc.vector.memset(pt[:, base:base + c0], 0.0)
                        nc.vector.tensor_mul(pt[:, base + c0:base + 512],
                                             pt[:, base + c0:base + 512],
                                             mask0[:, 0:512 - c0])

            def pv_unit(s):
                # attn for q-rows [s*128, (s+1)*128): 129-wide PV accumulation
                # (col 128 of VE is ones -> softmax denominator for free)
                nks = 4 * r + s + 1
                pv = pv_ps.tile([128, 129], F32, tag="pv", name="pv")
                for mk in range(nks):
                    nc.tensor.matmul(
                        pv[:], pts[mk // 2][:, (mk % 2) * 512 + s * 128:
                                            (mk % 2) * 512 + (s + 1) * 128],
                        VE[g][:, mk * 129:(mk + 1) * 129],
                        start=(mk == 0), stop=(mk == nks - 1))
                recip = small.tile([128, 1], F32, tag="recip", name="recip")
                nc.vector.reciprocal(recip[:], pv[:, 128:129])
                attn_q = small.tile([128, 128], BF16, tag="attnq", name="attnq")
                nc.vector.tensor_scalar_mul(attn_q[:], pv[:, 0:128], recip[:])
                tp = atr_ps.tile([128, 128], BF16, tag="atr", name="tp")
                nc.tensor.transpose(tp[:], attn_q[:], ident[:])
                nc.vector.tensor_copy(at_store[h][:, s * 128:(s + 1) * 128], tp[:])

            def start_pv():
                at_store[h] = attp.tile([128, 512], BF16, tag=f"at{h}", name=f"at{h}")

            units = [lambda p=p: pair_unit(p) for p in range(npair)]
            pv_units = [start_pv] + [lambda s=s: pv_unit(s) for s in range(4)]
            return units, pv_units

        def oproj_group(r_prev, s, j, at_prev, drain="v", pool=None):
            po = (pool or op_ps).tile([128, 512], F32, tag="op", name="po")
            for h in range(QH):
                nc.tensor.matmul(
                    po[:], at_prev[h][:, s * 128:(s + 1) * 128],
                    WO[h][:, j * 512:(j + 1) * 512],
                    start=(h == 0), stop=(h == QH - 1))
            ob = ost.tile([128, 512], F32, tag="ob", name="ob")
            if drain == "s":
                nc.scalar.copy(ob[:], po[:])
            else:
                nc.vector.tensor_copy(ob[:], po[:])
            row = r_prev * 512 + s * 128
            nc.sync.dma_start(out_d[row:row + 128, j * 512:(j + 1) * 512], ob[:])

        # o_proj deferred toward later chunks (longer softcap chains there)
        c_queue = []
        C_BUDGET = {0: 0, 1: 20, 2: 32, 3: 10 ** 6}

        inner = tc.tile_pool(name="sc_ps", bufs=1, space="PSUM")
        sc_ps = inner.__enter__()
        inner2 = tc.tile_pool(name="pv_ps", bufs=2, space="PSUM")
        pv_ps = inner2.__enter__()
        inner3 = tc.tile_pool(name="atr_ps", bufs=2, space="PSUM")
        atr_ps = inner3.__enter__()

        for r in range(NQ):
            # B stream: score pairs of head h interleaved with PV of head h-1
            b_units = []
            prev_pv = []
            for h in range(QH):
                units, pv_units = make_job(h, r)
                merged = []
                n = max(len(units), len(prev_pv))
                for i in range(n):
                    if i < len(units):
                        merged.append(units[i])
                    if i < len(prev_pv):
                        merged.append(prev_pv[i])
                b_units.extend(merged)
                prev_pv = pv_units
            b_units.extend(prev_pv)  # PV of the last head

            if r >= 1:
                at_prev = dict(at_store)
                for s in range(4):
                    for j in range(DOUT_CHUNKS):
                        c_queue.append(
                            lambda r=r, s=s, j=j, ap=at_prev:
                            oproj_group(r - 1, s, j, ap))
            n_c = min(C_BUDGET[r], len(c_queue))
            c_items = c_queue[:n_c]
            del c_queue[:n_c]

            n_slots = max(1, len(b_units) // 2)
            fi = 0
            slot = 0
            for i, u in enumerate(b_units):
                u()
                if i % 2 == 1:
                    slot += 1
                    want = (len(c_items) * slot) // n_slots
                    while fi < want:
                        c_items[fi]()
                        fi += 1
            while fi < len(c_items):
                c_items[fi]()
                fi += 1

        for c in c_queue:
            c()
        inner3.__exit__(None, None, None)
        inner2.__exit__(None, None, None)
        inner.__exit__(None, None, None)
        # epilogue: the freed PSUM banks give chunk-3's o_proj a 4-deep
        # pipeline (the 2-bank version ran at ~64% PE here)
        with tc.tile_pool(name="ep_ps", bufs=4, space="PSUM") as ep_ps:
            at_prev = dict(at_store)
            for s in range(4):
                for j in range(DOUT_CHUNKS):
                    oproj_group(NQ - 1, s, j, at_prev,
                                drain="s" if j % 2 else "v", pool=ep_ps)


_CACHED_NC = {}


def build(n_iters=1):
    if n_iters not in _CACHED_NC:
        nc = bacc.Bacc("TRN2", target_bir_lowering=False, debug=False)
        _emit(nc)
        nc.compile()
        _CACHED_NC[n_iters] = nc
    return _CACHED_NC[n_iters]


def host_tables():
    inv_freq = 1.0 / (ROPE_THETA ** (np.arange(0, HEAD_DIM, 2, dtype=np.float32) / HEAD_DIM))
    ang = np.arange(L, dtype=np.float32)[:, None] * inv_freq[None, :]  # [L, 64]
    cos, sin = np.cos(ang), np.sin(ang)
    cosT = np.concatenate([cos.T, cos.T], axis=0).astype(BF16_NP)
    sinT = np.concatenate([-sin.T, sin.T], axis=0).astype(BF16_NP)
    return np.ascontiguousarray(cosT), np.ascontiguousarray(sinT)


def host_mask():
    k = np.arange(128)[:, None]
    q = np.arange(512)[None, :]
    return np.ascontiguousarray((q >= k).astype(BF16_NP))


def make_in_maps(x, wq, wk, wv, wo):
    cosT, sinT = host_tables()
    mask = host_mask()
    # pre-tiled so each [128, 512] x tile is one contiguous 128KB DMA read
    xt = np.ascontiguousarray(
        x.reshape(L, D).T.astype(BF16_NP)
        .reshape(KC, 128, NQ, 512).transpose(2, 0, 1, 3)).reshape(NQ * KC * 128, 512)
    in_maps = []
    for c in range(N_CORES):
        qs = slice(c * QH * 128, (c + 1) * QH * 128)
        kvs = slice(c * KVH * 128, (c + 1) * KVH * 128)
        in_maps.append({
            "xt": xt,
            "wqt": np.ascontiguousarray(wq[qs].T.astype(BF16_NP)),
            "wkt": np.ascontiguousarray(wk[kvs].T.astype(BF16_NP)),
            "wvt": np.ascontiguousarray(wv[kvs].T.astype(BF16_NP)),
            "wot": np.ascontiguousarray(wo[:, qs].T.astype(BF16_NP)),
            "cost": cosT,
            "sint": sinT,
            "mask": mask,
        })
    return in_maps


def run(inputs, trace=False, trace_kwargs=None):
    from concourse.bass_utils import run_bass_kernel_spmd

    nc = build()
    x = np.asarray(inputs["x"], dtype=np.float32)
    in_maps = make_in_maps(
        x,
        np.asarray(inputs["wq"], dtype=np.float32),
        np.asarray(inputs["wk"], dtype=np.float32),
        np.asarray(inputs["wv"], dtype=np.float32),
        np.asarray(inputs["wo"], dtype=np.float32),
    )
    res = run_bass_kernel_spmd(
        nc, in_maps, core_ids=list(range(N_CORES)),
        trace=trace, **(trace_kwargs or {}))
    out = np.zeros((L, D), dtype=np.float32)
    for c in range(N_CORES):
        out += res.results[c]["out"]
    return out.reshape(x.shape), res


def kernel(**inputs) -> np.ndarray:
    out, _ = run(inputs, trace=False)
    return out


# revision 31
# speedup vs baseline: 1.0811x; 1.0811x over previous
"""Trainium2 Bass kernel for GQA attention (32 q heads / 16 kv heads, head_dim
128, L=2048, D=4608) with RoPE, tanh softcap 50, causal mask, o_proj.

Tensor-parallel over heads across 8 NeuronCores; core c owns q-heads 4c..4c+3
and kv-heads 2c..2c+1; host sums the 8 partial [L, D] outputs.

Phase 1: one pass over x columns per 512-wide chunk; 512-wide Q/K chains
(LDWEIGHTS fully hidden under the 213ns matmuls), 256-wide V chains; rope
applied during the PSUM drain; wq[k]/xc[k] DMAs interleaved so the first
chain starts ~10us in.  V carries a ones column per k-tile ([128, 16*129])
so the PV matmul accumulates the softmax denominator for free.

Phase 2 schedule (the win over the naive per-head loop, 611us -> 544us):
  - scores are computed in PAIRS of k-tiles: two 512-wide score MMs land in
    one 2-bank PSUM tile, then a single tanh and a single exp cover
    [128, 1024], halving the scalar-engine instruction count (its 352-cycle
    per-op overhead was a third of the softcap cost).  Diagonal tiles write
    q-aligned into the pair so one exp keeps columns aligned; tanh/exp of the
    stale PSUM in masked gaps is bounded and memset-zeroed afterwards.
  - PV for head h-1 (129-wide accumulation chains incl. the denominator
    column, then reciprocal/scale/PE-transpose) is interleaved unit-by-unit
    with head h's score pairs, so the PE always has ready matmuls while the
    scalar engine chews the tanh+exp backlog.
  - o_proj (s, j)-groups are drawn from a deferred queue with per-chunk
    budgets {0, 20, 32, rest}: later chunks have longer softcap chains, so
    more PE filler is reserved for them.
"""
import numpy as np
import ml_dtypes

import concourse.bass as bass
import concourse.mybir as mybir
import concourse.tile as tile
from concourse import bacc

F32 = mybir.dt.float32
BF16 = mybir.dt.bfloat16
BF16_NP = ml_dtypes.bfloat16
AF = mybir.ActivationFunctionType

N_HEADS = 32
N_KV = 16
HEAD_DIM = 128
ROPE_THETA = 10000.0
SOFTCAP = 50.0
SCALE = 1.0 / 12.0  # 1/sqrt(144)
L = 2048
D = 4608
N_CORES = 8
QH = N_HEADS // N_CORES        # 4 local q heads
KVH = N_KV // N_CORES          # 2 local kv heads
KC = D // 128                  # 36 contraction chunks
NQ = L // 512                  # 4 l-chunks of 512
LT = L // 128                  # 16 l-tiles of 128
DOUT_CHUNKS = D // 512         # 9 o_proj output chunks
PAIR_LAG = 1                   # attnT MM pair trails the exp by this many pairs


def _emit(nc):
    xt_d = nc.dram_tensor("xt", [NQ * KC * 128, 512], BF16, kind="ExternalInput")
    wqt_d = nc.dram_tensor("wqt", [D, QH * 128], BF16, kind="ExternalInput")
    wkt_d = nc.dram_tensor("wkt", [D, KVH * 128], BF16, kind="ExternalInput")
    wvt_d = nc.dram_tensor("wvt", [D, KVH * 128], BF16, kind="ExternalInput")
    wot_d = nc.dram_tensor("wot", [QH * 128, D], BF16, kind="ExternalInput")
    cost_d = nc.dram_tensor("cost", [128, L], BF16, kind="ExternalInput")
    sint_d = nc.dram_tensor("sint", [128, L], BF16, kind="ExternalInput")
    mask_d = nc.dram_tensor("mask", [128, 512], BF16, kind="ExternalInput")
    out_d = nc.dram_tensor("out", [L, D], F32, kind="ExternalOutput")

    with tile.TileContext(nc) as tc:
        with (
            tc.tile_pool(name="const", bufs=1) as const,
            tc.tile_pool(name="persist", bufs=1) as persist,
        ):
            ones = const.tile([128, 1], F32)
            nc.vector.memset(ones[:], 1.0)
            warm = const.tile([128, 2], F32)
            # preload the exp/tanh activation table set during phase 1
            nc.scalar.activation(warm[:, 0:1], ones[:], AF.Tanh)
            nc.scalar.activation(warm[:, 1:2], warm[:, 0:1], AF.Exp)
            cost = const.tile([128, L], BF16)
            sint = const.tile([128, L], BF16)
            mask0 = const.tile([128, 512], BF16)

            # persistent per-head tensors
            QT = [persist.tile([128, L], BF16, tag=f"qt{h}", name=f"qt{h}") for h in range(QH)]
            KT = [persist.tile([128, L], BF16, tag=f"kt{g}", name=f"kt{g}") for g in range(KVH)]
            # V extended with a ones column per k-tile: [128, 16*129]
            V = [persist.tile([128, LT * 129], BF16, tag=f"v{g}", name=f"v{g}") for g in range(KVH)]

            _phase1(nc, tc, cost, sint, mask0,
                    dict(xt=xt_d, wqt=wqt_d, wkt=wkt_d, wvt=wvt_d,
                         cost=cost_d, sint=sint_d, mask=mask_d), QT, KT, V)
            _phase2(nc, tc, ones, mask0, QT, KT, V, wot_d, out_d)
    return nc


def _phase1(nc, tc, cost, sint, mask0, dram, QT, KT, V):
    def drain_rope(rtmp, ps, dst, nq, pj_free):
        """psum [128,512] f32 -> rope -> dst bf16 [128,512] slice."""
        cols = slice(nq * 512, (nq + 1) * 512)
        raw = rtmp.tile([128, 512], F32, tag="raw", name="raw")
        nc.scalar.activation(raw[:], ps[:], AF.Copy)
        swap = rtmp.tile([128, 512], F32, tag="swap", name="swap")
        nc.scalar.activation(swap[0:64, :], ps[64:128, :], AF.Copy)
        nc.scalar.activation(swap[64:128, :], ps[0:64, :], AF.Copy)
        nc.vector.tensor_mul(raw[:], raw[:], cost[:, cols])
        nc.vector.tensor_mul(swap[:], swap[:], sint[:, cols])
        nc.vector.tensor_add(dst[:, cols], raw[:], swap[:])

    with (
        tc.tile_pool(name="xcol", bufs=2) as xcol,
        tc.tile_pool(name="rtmp", bufs=3) as rtmp,
        tc.tile_pool(name="wts", bufs=1) as wts,
        tc.tile_pool(name="pj_psum", bufs=2, space="PSUM") as pj_psum,
        tc.tile_pool(name="warm_ps", bufs=1, space="PSUM") as warm_ps,
    ):
        # single pass over x columns computing Q, K (rope'd, [d, l]) and V
        # (direct [l, d] with xT stationary) per 512-wide chunk.
        wq, wk, wv, xc0 = [], [], [], []
        for k in range(KC):
            w = wts.tile([128, QH * 128], BF16, tag=f"q{k}", name=f"wq{k}")
            nc.sync.dma_start(w[:], dram["wqt"][k * 128:(k + 1) * 128, :])
            wq.append(w)
            t = xcol.tile([128, 512], BF16, tag=f"x{k}", name=f"xc{k}")
            nc.sync.dma_start(t[:], dram["xt"][k * 128:(k + 1) * 128, :])
            xc0.append(t)
            w = wts.tile([128, KVH * 128], BF16, tag=f"k{k}", name=f"wk{k}")
            nc.gpsimd.dma_start(w[:], dram["wkt"][k * 128:(k + 1) * 128, :])
            wk.append(w)
        nc.gpsimd.dma_start(cost[:], dram["cost"][:])
        nc.gpsimd.dma_start(sint[:], dram["sint"][:])
        for k in range(KC):
            w = wts.tile([128, KVH * 128], BF16, tag=f"v{k}", name=f"wv{k}")
            nc.gpsimd.dma_start(w[:], dram["wvt"][k * 128:(k + 1) * 128, :])
            wv.append(w)
        nc.gpsimd.dma_start(mask0[:], dram["mask"][:])

        for nq in range(NQ):
            if nq == 0:
                # k-outer warmup: all 6 Q/K chains accumulate concurrently in
                # 6 PSUM banks, so each arriving (wq, x, wk) DMA trio feeds 6
                # matmuls - the cold-start is compute- not DMA-bound.
                xc = xc0
                wps = [warm_ps.tile([128, 512], F32, tag=f"w{i}", name=f"wps{i}")
                       for i in range(QH + KVH)]
                for k in range(KC):
                    for h in range(QH):
                        nc.tensor.matmul(
                            wps[h][:], wq[k][:, h * 128:(h + 1) * 128], xc[k][:],
                            start=(k == 0), stop=(k == KC - 1),
                            skip_group_check=True)
                    for g in range(KVH):
                        nc.tensor.matmul(
                            wps[QH + g][:], wk[k][:, g * 128:(g + 1) * 128], xc[k][:],
                            start=(k == 0), stop=(k == KC - 1),
                            skip_group_check=True)
                for h in range(QH):
                    drain_rope(rtmp, wps[h], QT[h], 0, pj_psum)
                for g in range(KVH):
                    drain_rope(rtmp, wps[QH + g], KT[g], 0, pj_psum)
            else:
                xc = []
                for k in range(KC):
                    t = xcol.tile([128, 512], BF16, tag=f"x{k}", name=f"xc{k}")
                    row = (nq * KC + k) * 128
                    nc.gpsimd.dma_start(t[:], dram["xt"][row:row + 128, :])
                    xc.append(t)
                for h in range(QH):
                    ps = pj_psum.tile([128, 512], F32, tag="qk", name="ps")
                    for k in range(KC):
                        nc.tensor.matmul(
                            ps[:], wq[k][:, h * 128:(h + 1) * 128], xc[k][:],
                            start=(k == 0), stop=(k == KC - 1))
                    drain_rope(rtmp, ps, QT[h], nq, pj_psum)
                for g in range(KVH):
                    ps = pj_psum.tile([128, 512], F32, tag="qk", name="ps")
                    for k in range(KC):
                        nc.tensor.matmul(
                            ps[:], wk[k][:, g * 128:(g + 1) * 128], xc[k][:],
                            start=(k == 0), stop=(k == KC - 1))
                    drain_rope(rtmp, ps, KT[g], nq, pj_psum)
            for sub in range(4):
                mk = nq * 4 + sub
                ps = pj_psum.tile([128, 512], F32, tag="qk", name="ps")
                for k in range(KC):
                    nc.tensor.matmul(
                        ps[:, 0:KVH * 128], xc[k][:, sub * 128:(sub + 1) * 128],
                        wv[k][:], start=(k == 0), stop=(k == KC - 1))
                for g in range(KVH):
                    nc.vector.tensor_copy(
                        V[g][:, mk * 129:mk * 129 + 128],
                        ps[:, g * 128:(g + 1) * 128])
                    nc.vector.memset(
                        V[g][:, mk * 129 + 128:mk * 129 + 129], 1.0)


def _phase2(nc, tc, ones, mask0, QT, KT, VE, wot_d, out_d):
    from concourse.masks import make_identity

    with (
        tc.tile_pool(name="wo", bufs=1) as wop,
        tc.tile_pool(name="ident", bufs=1) as idp,
        tc.tile_pool(name="ttp", bufs=2) as ttp,
        tc.tile_pool(name="ptp", bufs=16) as ptp,
        tc.tile_pool(name="attnt", bufs=3) as attp,
        tc.tile_pool(name="small", bufs=4) as small,
        tc.tile_pool(name="ost", bufs=6) as ost,
        tc.tile_pool(name="op_ps", bufs=2, space="PSUM") as op_ps,
    ):
        ident = idp.tile([128, 128], BF16)
        make_identity(nc, ident[:])
        WO = []
        for h in range(QH):
            w = wop.tile([128, D], BF16, tag=f"wo{h}")
            nc.sync.dma_start(w[:], wot_d[h * 128:(h + 1) * 128, :])
            WO.append(w)

        at_store = {}

        def make_job(h, r):
            """Closures for one (head, chunk): score pairs then 4 PV units."""
            g = h // 2
            nkt = 4 * r + 4
            npair = nkt // 2
            qt = QT[h]
            q0 = r * 512
            pts = {}

            def pair_unit(p):
                sc = sc_ps.tile([128, 1024], F32, tag="sc", name="sc")
                c0s = []
                for i in range(2):
                    mk = 2 * p + i
                    o = mk - 4 * r
                    c0 = max(0, o) * 128
                    c0s.append(c0)
                    nc.tensor.matmul(sc[:, i * 512 + c0:(i + 1) * 512],
                                     KT[g][:, mk * 128:(mk + 1) * 128],
                                     qt[:, q0 + c0:q0 + 512],
                                     start=True, stop=True)
                tt = ttp.tile([128, 1024], F32, tag="tt", name="tt")
                pt = ptp.tile([128, 1024], BF16, tag="pt", name="pt")
                pts[p] = pt
                if c0s[0] + c0s[1] > 424:
                    # heavily-masked diagonal pair: exact-width ops beat one
                    # batched op (352-cycle ACT overhead vs dead columns)
                    for i in range(2):
                        a, b = i * 512 + c0s[i], (i + 1) * 512
                        nc.scalar.activation(tt[:, a:b], sc[:, a:b], AF.Tanh,
                                             scale=SCALE / SOFTCAP)
                        nc.scalar.activation(pt[:, a:b], tt[:, a:b], AF.Exp,
                                             scale=SOFTCAP)
                else:
                    # tanh of stale psum in masked gaps is bounded; its exp is
                    # finite and the memset below zeroes it.
                    nc.scalar.activation(tt[:], sc[:], AF.Tanh, scale=SCALE / SOFTCAP)
                    nc.scalar.activation(pt[:], tt[:], AF.Exp, scale=SOFTCAP)
                for i in range(2):
                    mk = 2 * p + i
                    o = mk - 4 * r
                    c0 = max(0, o) * 128
                    base = i * 512
                    if o >= 0:
                        if c0 > 0:
                            nc.vector.memset(pt[:, base:base + c0], 0.0)
                        nc.vector.tensor_mul(pt[:, base + c0:base + 512],
                                             pt[:, base + c0:base + 512],
                                             mask0[:, 0:512 - c0])

            def pv_unit(s):
                # attn for q-rows [s*128, (s+1)*128): 129-wide PV accumulation
                # (col 128 of VE is ones -> softmax denominator for free)
                nks = 4 * r + s + 1
                pv = pv_ps.tile([128, 129], F32, tag="pv", name="pv")
                for mk in range(nks):
                    nc.tensor.matmul(
                        pv[:], pts[mk // 2][:, (mk % 2) * 512 + s * 128:
                                            (mk % 2) * 512 + (s + 1) * 128],
                        VE[g][:, mk * 129:(mk + 1) * 129],
                        start=(mk == 0), stop=(mk == nks - 1))
                recip = small.tile([128, 1], F32, tag="recip", name="recip")
                nc.vector.reciprocal(recip[:], pv[:, 128:129])
                attn_q = small.tile([128, 128], BF16, tag="attnq", name="attnq")
                nc.vector.tensor_scalar_mul(attn_q[:], pv[:, 0:128], recip[:])
                tp = atr_ps.tile([128, 128], BF16, tag="atr", name="tp")
                nc.tensor.transpose(tp[:], attn_q[:], ident[:])
                nc.vector.tensor_copy(at_store[h][:, s * 128:(s + 1) * 128], tp[:])

            def start_pv():
                at_store[h] = attp.tile([128, 512], BF16, tag=f"at{h}", name=f"at{h}")

            units = [lambda p=p: pair_unit(p) for p in range(npair)]
            pv_units = [start_pv] + [lambda s=s: pv_unit(s) for s in range(4)]
            return units, pv_units

        def oproj_group(r_prev, s, j, at_prev, drain="v", pool=None):
            po = (pool or op_ps).tile([128, 512], F32, tag="op", name="po")
            for h in range(QH):
                nc.tensor.matmul(
                    po[:], at_prev[h][:, s * 128:(s + 1) * 128],
                    WO[h][:, j * 512:(j + 1) * 512],
                    start=(h == 0), stop=(h == QH - 1))
            ob = ost.tile([128, 512], F32, tag="ob", name="ob")
            if drain == "s":
                nc.scalar.copy(ob[:], po[:])
            else:
                nc.vector.tensor_copy(ob[:], po[:])
            row = r_prev * 512 + s * 128
            nc.sync.dma_start(out_d[row:row + 128, j * 512:(j + 1) * 512], ob[:])

        # o_proj deferred toward later chunks (longer softcap chains there)
        c_queue = []
        C_BUDGET = {0: 0, 1: 20, 2: 32, 3: 10 ** 6}

        inner = tc.tile_pool(name="sc_ps", bufs=1, space="PSUM")
        sc_ps = inner.__enter__()
        inner2 = tc.tile_pool(name="pv_ps", bufs=2, space="PSUM")
        pv_ps = inner2.__enter__()
        inner3 = tc.tile_pool(name="atr_ps", bufs=2, space="PSUM")
        atr_ps = inner3.__enter__()

        for r in range(NQ):
            # B stream: score pairs of head h interleaved with PV of head h-1
            b_units = []
            prev_pv = []
            for h in range(QH):
                units, pv_units = make_job(h, r)
                merged = []
                n = max(len(units), len(prev_pv))
                for i in range(n):
                    if i < len(units):
                        merged.append(units[i])
                    if i < len(prev_pv):
                        merged.append(prev_pv[i])
                b_units.extend(merged)
                prev_pv = pv_units
            b_units.extend(prev_pv)  # PV of the last head

            if r >= 1:
                at_prev = dict(at_store)
                for s in range(4):
                    for j in range(DOUT_CHUNKS):
                        c_queue.append(
                            lambda r=r, s=s, j=j, ap=at_prev:
                            oproj_group(r - 1, s, j, ap))
            n_c = min(C_BUDGET[r], len(c_queue))
            c_items = c_queue[:n_c]
            del c_queue[:n_c]

            n_slots = max(1, len(b_units) // 2)
            fi = 0
            slot = 0
            for i, u in enumerate(b_units):
                u()
                if i % 2 == 1:
                    slot += 1
                    want = (len(c_items) * slot) // n_slots
                    while fi < want:
                        c_items[fi]()
                        fi += 1
            while fi < len(c_items):
                c_items[fi]()
                fi += 1

        for c in c_queue:
            c()
        inner3.__exit__(None, None, None)
        inner2.__exit__(None, None, None)
        inner.__exit__(None, None, None)
        # epilogue: the freed PSUM banks give chunk-3's o_proj a 4-deep
        # pipeline (the 2-bank version ran at ~64% PE here)
        with tc.tile_pool(name="ep_ps", bufs=4, space="PSUM") as ep_ps:
            at_prev = dict(at_store)
            for s in range(4):
                for j in range(DOUT_CHUNKS):
                    oproj_group(NQ - 1, s, j, at_prev,
                                drain="s" if j % 2 else "v", pool=ep_ps)


_CACHED_NC = {}


def build(n_iters=1):
    if n_iters not in _CACHED_NC:
        nc = bacc.Bacc("TRN2", target_bir_lowering=False, debug=False)
        _emit(nc)
        nc.compile()
        _CACHED_NC[n_iters] = nc
    return _CACHED_NC[n_iters]


def host_tables():
    inv_freq = 1.0 / (ROPE_THETA ** (np.arange(0, HEAD_DIM, 2, dtype=np.float32) / HEAD_DIM))
    ang = np.arange(L, dtype=np.float32)[:, None] * inv_freq[None, :]  # [L, 64]
    cos, sin = np.cos(ang), np.sin(ang)
    cosT = np.concatenate([cos.T, cos.T], axis=0).astype(BF16_NP)
    sinT = np.concatenate([-sin.T, sin.T], axis=0).astype(BF16_NP)
    return np.ascontiguousarray(cosT), np.ascontiguousarray(sinT)


def host_mask():
    k = np.arange(128)[:, None]
    q = np.arange(512)[None, :]
    return np.ascontiguousarray((q >= k).astype(BF16_NP))


def make_in_maps(x, wq, wk, wv, wo):
    cosT, sinT = host_tables()
    mask = host_mask()
    # pre-tiled so each [128, 512] x tile is one contiguous 128KB DMA read
    xt = np.ascontiguousarray(
        x.reshape(L, D).T.astype(BF16_NP)
        .reshape(KC, 128, NQ, 512).transpose(2, 0, 1, 3)).reshape(NQ * KC * 128, 512)
    in_maps = []
    for c in range(N_CORES):
        qs = slice(c * QH * 128, (c + 1) * QH * 128)
        kvs = slice(c * KVH * 128, (c + 1) * KVH * 128)
        in_maps.append({
            "xt": xt,
            "wqt": np.ascontiguousarray(wq[qs].T.astype(BF16_NP)),
            "wkt": np.ascontiguousarray(wk[kvs].T.astype(BF16_NP)),
            "wvt": np.ascontiguousarray(wv[kvs].T.astype(BF16_NP)),
            "wot": np.ascontiguousarray(wo[:, qs].T.astype(BF16_NP)),
            "cost": cosT,
            "sint": sinT,
            "mask": mask,
        })
    return in_maps


def run(inputs, trace=False, trace_kwargs=None):
    from concourse.bass_utils import run_bass_kernel_spmd

    nc = build()
    x = np.asarray(inputs["x"], dtype=np.float32)
    in_maps = make_in_maps(
        x,
        np.asarray(inputs["wq"], dtype=np.float32),
        np.asarray(inputs["wk"], dtype=np.float32),
        np.asarray(inputs["wv"], dtype=np.float32),
        np.asarray(inputs["wo"], dtype=np.float32),
    )
    res = run_bass_kernel_spmd(
        nc, in_maps, core_ids=list(range(N_CORES)),
        trace=trace, **(trace_kwargs or {}))
    out = np.zeros((L, D), dtype=np.float32)
    for c in range(N_CORES):
        out += res.results[c]["out"]
    return out.reshape(x.shape), res


def kernel(**inputs) -> np.ndarray:
    out, _ = run(inputs, trace=False)
    return out
